# revision 57
# baseline (speedup 1.0000x reference)
"""Trainium2 Bass kernel for BertSelfAttention (B=1, S=4096, HID=768, 12 heads).

Sharding: 8 cores = 4 head-groups x 2 query-halves. Each core computes 3 heads
for 2048 query rows against all 4096 keys, fused (scores never hit HBM).

v3 design (vs v1 baseline):
  - Two head-blocks are processed as a PAIR, key chunk by key chunk: one
    [128,1024] fp32 PSUM score tile holds BOTH blocks' scores for a chunk
    (lo block cols 0:512, hi block 512:1024) so ONE ScalarE exp per chunk
    covers both blocks. ScalarE (the bottleneck engine, ~1.15us per exp)
    runs back-to-back with zero idle in steady state.
  - Heads 0/1 pair per query block; head 2 pairs with itself across two
    query blocks. Score matmuls stay plain full-128 contractions with
    zero-padded halves: a row-tiled 64x128 variant was measured FASTER on
    the PE but pushed the chip into the P0 power state (all clocks x5/6),
    slowing the bottleneck ScalarE - zero halves toggle nothing and are
    power-cheap.
  - hsqT input dropped: per-core hsT is key-permuted so the core's own query
    rows are always columns 0:2048 (softmax is permutation-invariant over
    keys); q-projections just index hsT.
  - PSUM: sc 2x2 banks + cx_lo + cx_hi + 2 proj banks = 8 exactly.
  - kt/qt zero halves are written just-in-time by small DVE memsets per
    projection unit (no big ramp memsets).

Per-core dataflow otherwise follows v1: bf16 matmuls, fp32 PSUM, additive
mask handled by scaling V rows (and the appended ones-column) with exp(mask),
V augmented with a ones column per head so the context matmul accumulates the
softmax denominator for free, ctx^T tiles PE-transposed back to [q, d] and
divided by the denominator on VectorE.
"""

import sys

sys.path.insert(0, "/opt/trn_rl_repo")

import ml_dtypes
import numpy as np

import concourse.bacc as bacc
import concourse.mybir as mybir
import concourse.tile as tile
from concourse import bass_utils

B, S, HID = 1, 4096, 768
NH, HD = 12, 64
N_CORES = 8
HG = 4  # head-groups (tensor parallel)
QS = 2  # query splits (data parallel on sequence)
HPC = NH // HG  # 3 heads per core
SQ = S // QS  # 2048 query rows per core
CC = HPC * HD  # 192 projection columns per core
WCC = 256  # weight cols per chunk in wqb/wkb: [h0|h1|h2|h2]
VC = HPC * (HD + 1)  # 195 augmented V columns (ones col per head)
NHC = HID // 128  # 6 contraction chunks
NT = S // 128  # 32 key tiles
NJ = SQ // 512  # 4 query blocks

f32 = mybir.dt.float32
bf16 = mybir.dt.bfloat16
bf16np = ml_dtypes.bfloat16

# pair-blocks: (h_lo, h_hi, j_lo, j_hi) — heads 0/1 pair per query block,
# head 2 pairs with itself across two query blocks
PBS = [(0, 1, j, j) for j in range(NJ)] + [(2, 2, 0, 1), (2, 2, 2, 3)]

_CACHE = {}


def _build():
    EXP = mybir.ActivationFunctionType.Exp
    nc = bacc.Bacc("TRN2", target_bir_lowering=False)

    # hsT is block-interleaved host-side: [128 partitions, 8 query/key-column
    # blocks x (6 hid-chunks x 512 cols)] so every DMA slice is one fully
    # contiguous per-partition run (max packet size, ~10x queue throughput
    # vs the naive [HID, S] layout whose runs were 1KB strided)
    hsT_d = nc.dram_tensor("hsT", [128, NHC * S], bf16, kind="ExternalInput")
    wqb_d = nc.dram_tensor("wqb", [128, NHC * WCC], bf16, kind="ExternalInput")
    wkb_d = nc.dram_tensor("wkb", [128, NHC * WCC], bf16, kind="ExternalInput")
    wvb_d = nc.dram_tensor("wvb", [128, NHC * VC], bf16, kind="ExternalInput")
    bqt_d = nc.dram_tensor("bqt", [128, HPC], f32, kind="ExternalInput")
    bkt_d = nc.dram_tensor("bkt", [128, HPC], f32, kind="ExternalInput")
    bvb_d = nc.dram_tensor("bvb", [128, VC], bf16, kind="ExternalInput")
    bones_d = nc.dram_tensor("bones", [128, 128], bf16, kind="ExternalInput")
    maskt_d = nc.dram_tensor("maskt", [128, NT], f32, kind="ExternalInput")
    ident_d = nc.dram_tensor("ident", [128, 128], f32, kind="ExternalInput")
    # one contiguous [128, 256] slab per out-stage (2 per pair-block); the
    # host undoes the layout. Contiguous runs keep the out DMAs off the
    # slow small-packet path.
    out_d = nc.dram_tensor("out", [2 * len(PBS) * 128, 4 * 64], f32,
                           kind="ExternalOutput")

    with tile.TileContext(nc) as tc:
        with (
            tc.tile_pool(name="persist", bufs=1) as P,
            tc.tile_pool(name="work", bufs=4) as WK,
            tc.tile_pool(name="outp", bufs=2) as OP,
            tc.tile_pool(name="scp", bufs=2, space="PSUM") as SCP,
            tc.tile_pool(name="cxp", bufs=1, space="PSUM") as CP,
            tc.tile_pool(name="ppsum", bufs=1, space="PSUM") as PP,
        ):
            # ---- persistent SBUF tensors ----
            # chunk-major transposed activations: chunk c at cols [c*S, (c+1)*S)
            hsT = P.tile([128, NHC * S], bf16, tag="hsT")
            wqb = P.tile([128, NHC * WCC], bf16, tag="wqb")
            wkb = P.tile([128, NHC * WCC], bf16, tag="wkb")
            wvb = P.tile([128, NHC * VC], bf16, tag="wvb")
            bvb = P.tile([128, VC], bf16, tag="bvb")
            bones = P.tile([128, 128], bf16, tag="bones")
            bqt = P.tile([128, HPC], f32, tag="bqt")
            bkt = P.tile([128, HPC], f32, tag="bkt")
            maskt = P.tile([128, NT], f32, tag="maskt")
            wmask = P.tile([128, NT], f32, tag="wmask")
            identf = P.tile([128, 128], f32, tag="identf")
            # per-head K^T/Q^T: head h occupies partitions H_LO[h]:H_LO[h]+64,
            # the other half is zero (written by the zero-padded projection,
            # no memsets) so full-128 score contractions are exact and
            # power-cheap (the zero half toggles nothing in the PE array)
            kts = [
                P.tile([128, S], bf16, tag=f"kt{h}", name=f"kt{h}")
                for h in range(HPC)
            ]
            qts = [
                P.tile([128, SQ], bf16, tag=f"qt{h}", name=f"qt{h}")
                for h in range(HPC)
            ]
            vv = P.tile([128, NT * VC], bf16, tag="vv")

            # ---- DMA helpers ----
            HB = NHC * 512  # one 512-col block of all 6 chunks

            def load_hsT_block(b, queue="sync"):
                eng = nc.sync if queue == "sync" else nc.scalar
                eng.dma_start(
                    hsT[:, b * HB : (b + 1) * HB], hsT_d[:, b * HB : (b + 1) * HB]
                )

            # ---- q/k projection units ----
            # one paired matmul chain produces both partition halves:
            # pair 0 -> stationary cols 0:128 of each chunk ([h0|h1]),
            # pair 1 -> cols 128:256 ([h2|h2])
            def emit_qk_mm(kind, pi, j, c, ps):
                wsrc = wqb if kind == "qt" else wkb
                coff = 128 * pi
                nc.tensor.matmul(
                    ps[:],
                    wsrc[:, c * WCC + coff : c * WCC + coff + 128],
                    hsT[:, j * HB + c * 512 : j * HB + (c + 1) * 512],
                    start=(c == 0),
                    stop=(c == NHC - 1),
                )

            def emit_qk_finish(kind, pi, j, ps):
                dsts = qts if kind == "qt" else kts
                bias = bqt if kind == "qt" else bkt
                blk = slice(j * 512, (j + 1) * 512)
                if pi == 0:
                    nc.vector.tensor_scalar_add(
                        dsts[0][0:64, blk], ps[0:64, :], bias[0:64, 0:1]
                    )
                    nc.vector.tensor_scalar_add(
                        dsts[1][64:128, blk], ps[64:128, :], bias[64:128, 1:2]
                    )
                    # zero the complementary halves just-in-time (power-cheap
                    # full-128 contraction needs them zero)
                    nc.vector.memset(dsts[0][64:128, blk], 0.0)
                    nc.vector.memset(dsts[1][0:64, blk], 0.0)
                else:
                    # h2: upper weight cols are zero-padded, so the upper add
                    # writes zeros (+ zero bias) — both halves covered
                    nc.vector.tensor_scalar_add(
                        dsts[2][0:64, blk], ps[0:64, :], bias[0:64, 2:3]
                    )
                    nc.vector.tensor_scalar_add(
                        dsts[2][64:128, blk], ps[64:128, :], bias[64:128, 2:3]
                    )

            def qk_unit(kind, pi, j):
                ps = PP.tile([128, 512], f32, tag="ps", name="ps")
                for c in range(NHC):
                    emit_qk_mm(kind, pi, j, c, ps)
                emit_qk_finish(kind, pi, j, ps)

            # stepwise projection queue: one matmul per call so bursts never
            # overrun the per-tile PE slack
            proj_q = []

            def enqueue_proj(kind, pi, j):
                proj_q.append({"kind": kind, "pi": pi, "j": j, "step": 0})

            def proj_step():
                if not proj_q:
                    return
                st = proj_q[0]
                c = st["step"]
                if c == 0:
                    st["ps"] = PP.tile([128, 512], f32, tag="ps", name="ps")
                emit_qk_mm(st["kind"], st["pi"], st["j"], c, st["ps"])
                if c == NHC - 1:
                    emit_qk_finish(st["kind"], st["pi"], st["j"], st["ps"])
                    proj_q.pop(0)
                else:
                    st["step"] += 1

            def v_unit(t):
                pv = PP.tile([128, VC], f32, tag="pv", name="pv")
                base = (t // 4) * HB + (t % 4) * 128
                for c in range(NHC):
                    nc.tensor.matmul(
                        pv[:],
                        hsT[:, base + c * 512 : base + c * 512 + 128],
                        wvb[:, c * VC : (c + 1) * VC],
                        start=(c == 0),
                        stop=False,
                    )
                # bias add via row-0-selector stationary: full-128 operands so
                # the PE never leaves 128x128 tiling mode
                nc.tensor.matmul(pv[:], bones[:], bvb[:], start=False, stop=True)
                nc.vector.tensor_scalar_mul(
                    vv[:, t * VC : (t + 1) * VC], pv[:], wmask[:, t : t + 1]
                )

            # ---- deferred out-stage, pipelined into the next block ----
            out_stage_q = []

            def emit_out_stage():
                if not out_stage_q:
                    return
                # prioritize step-0 (the DVE copy that frees the cx PSUM
                # bank) of every queued entry, so the next block's ctx
                # accumulation never waits long on the bank
                entry = None
                for e in out_stage_q:
                    if e[3]["step"] == 0:
                        entry = e
                        break
                if entry is None:
                    entry = out_stage_q[0]
                _advance_out_stage(entry)

            def _advance_out_stage(entry):
                jq, h, cx, st = entry
                if st["step"] == 0:
                    cs = OP.tile([65, 512], f32, tag="cs", name="cs")
                    nc.vector.tensor_copy(cs[:], cx[:])
                    st["cs"] = cs
                    st["ot"] = OP.tile([128, 4 * 64], f32, tag="ot", name="ot")
                elif st["step"] == 1:
                    cs = st["cs"]
                    tp2 = PP.tile(
                        [128, 4 * 65], f32, tag=st.get("tag", "pv"), name="tp2"
                    )
                    st["tp2"] = tp2
                    for t4 in range(4):
                        nc.tensor.transpose(
                            tp2[:, t4 * 65 : (t4 + 1) * 65],
                            cs[:, t4 * 128 : (t4 + 1) * 128],
                            identf[0:65, 0:65],
                        )
                elif st["step"] == 2:
                    tp2 = st["tp2"]
                    # one strided reciprocal covers all four denominator cols
                    rc = OP.tile([128, 4], f32, tag="rc", name="rc")
                    den = tp2.rearrange("p (t c) -> p t c", c=65)[:, :, 64:65]
                    nc.vector.reciprocal(rc.rearrange("p (t c) -> p t c", c=1), den)
                    st["rc"] = rc
                    nc.vector.tensor_scalar_mul(
                        st["ot"][:, 0:64], tp2[:, 0:64], rc[:, 0:1]
                    )
                    nc.vector.tensor_scalar_mul(
                        st["ot"][:, 64:128], tp2[:, 65:129], rc[:, 1:2]
                    )
                elif st["step"] == 3:
                    tp2, ot, rc = st["tp2"], st["ot"], st["rc"]
                    nc.vector.tensor_scalar_mul(
                        ot[:, 128:192], tp2[:, 130:194], rc[:, 2:3]
                    )
                    nc.vector.tensor_scalar_mul(
                        ot[:, 192:256], tp2[:, 195:259], rc[:, 3:4]
                    )
                    si = st["si"]
                    # the final pair's outputs go out on the fast scalar
                    # queue (ScalarE is idle by then); mid-kernel stages use
                    # sync so DMA pushes never occupy the bottleneck engine
                    eng = nc.scalar if st.get("tag") == "ps" else nc.sync
                    eng.dma_start(out_d[si * 128 : (si + 1) * 128, :], ot[:])
                    for idx, e in enumerate(out_stage_q):
                        if e[3] is st:
                            del out_stage_q[idx]
                            break
                    return
                st["step"] += 1

            def flush_out_stages():
                # round-robin so the two final out-stages (on separate PSUM
                # slots) overlap across engines
                while out_stage_q:
                    for e in list(out_stage_q):
                        _advance_out_stage(e)

            # ---- ramp: pipelined input loads + first-needed projections ----
            # mask load + exp first: ScalarE is in-order, so this tiny
            # ACTIVATE must clear the queue before the first score exp
            # minimal ramp: only what gates the first score exp. Everything
            # else is JIT inside pair-block 0, where the activation stream
            # covers ~1.1us of PE work per tile; ramp work has zero overlap.
            # the first hsT block rides the scalar HWDGE queue ahead of the
            # mask activation so it lands as early as possible
            load_hsT_block(0, queue="scalar")
            load_hsT_block(1, queue="scalar")
            nc.sync.dma_start(maskt[:], maskt_d[:])
            nc.sync.dma_start(wqb[:], wqb_d[:])
            nc.sync.dma_start(bqt[:], bqt_d[:])
            nc.sync.dma_start(wkb[:], wkb_d[:])
            nc.sync.dma_start(bkt[:], bkt_d[:])
            nc.scalar.activation(wmask[:], maskt[:], EXP)
            nc.sync.dma_start(wvb[:], wvb_d[:])
            nc.sync.dma_start(bvb[:], bvb_d[:])
            nc.sync.dma_start(bones[:], bones_d[:])
            nc.sync.dma_start(identf[:], ident_d[:])
            load_hsT_block(2)
            load_hsT_block(3)
            # PE warm-up right before the first projections: dummy matmuls
            # (gated only on the small wqb DMA) open the HAM clock gate so
            # the projections chain into real work at 2.4 GHz
            warm = PP.tile([128, 512], f32, tag="ps", name="warm")
            for i in range(8):
                nc.tensor.matmul(
                    warm[:],
                    wqb[:, 0:128],
                    wqb[:, 512:1024],
                    start=True,
                    stop=True,
                )
            qk_unit("qt", 0, 0)
            # kt block 0 in two pieces: keys 0:128 first, so the first score
            # matmul (which only needs key chunk 0) fires ~2us earlier
            ps0 = PP.tile([128, 512], f32, tag="ps", name="ps0")
            for piece in ((0, 128), (128, 512)):
                p0, p1 = piece
                for c in range(NHC):
                    nc.tensor.matmul(
                        ps0[:, p0:p1],
                        wkb[:, c * WCC : c * WCC + 128],
                        hsT[:, c * 512 + p0 : c * 512 + p1],
                        start=(c == 0),
                        stop=(c == NHC - 1),
                    )
                nc.vector.tensor_scalar_add(
                    kts[0][0:64, p0:p1], ps0[0:64, p0:p1], bkt[0:64, 0:1]
                )
                nc.vector.tensor_scalar_add(
                    kts[1][64:128, p0:p1], ps0[64:128, p0:p1], bkt[64:128, 1:2]
                )
                nc.vector.memset(kts[0][64:128, p0:p1], 0.0)
                nc.vector.memset(kts[1][0:64, p0:p1], 0.0)
            load_hsT_block(4, queue="scalar")
            load_hsT_block(5, queue="scalar")
            load_hsT_block(6)
            load_hsT_block(7)

            # per-pair-block projection enqueue schedule (ready just in time)
            pb_enqueue = {
                0: [("kt", 0, j) for j in range(1, 8)] + [("qt", 0, 1)],
                1: [("qt", 0, 2)] + [("kt", 1, j) for j in range(4)],
                2: [("qt", 0, 3)] + [("kt", 1, j) for j in range(4, 8)],
                3: [("qt", 1, 0), ("qt", 1, 1)],
                4: [("qt", 1, 2), ("qt", 1, 3)],
            }

            pending_final = None

            for pb_idx, (h_lo, h_hi, j_lo, j_hi) in enumerate(PBS):
                for item in pb_enqueue.get(pb_idx, []):
                    enqueue_proj(*item)
                cx_lo = CP.tile([65, 512], f32, tag="cxlo", name="cxlo")
                cx_hi = CP.tile([65, 512], f32, tag="cxhi", name="cxhi")
                pts = []

                def emit_ctx(g, pts=pts, cx_lo=cx_lo, cx_hi=cx_hi,
                             h_lo=h_lo, h_hi=h_hi):
                    pt = pts[g]
                    nc.tensor.matmul(
                        cx_lo[:],
                        vv[:, g * VC + h_lo * 65 : g * VC + h_lo * 65 + 65],
                        pt[:, 0:512],
                        start=(g == 0),
                        stop=(g == NT - 1),
                    )
                    nc.tensor.matmul(
                        cx_hi[:],
                        vv[:, g * VC + h_hi * 65 : g * VC + h_hi * 65 + 65],
                        pt[:, 512:1024],
                        start=(g == 0),
                        stop=(g == NT - 1),
                    )

                for t in range(NT):
                    # scores for key chunk t, both paired head-blocks, as
                    # plain full-128 contractions (zero halves contribute
                    # nothing): lo block -> cols 0:512, hi block -> 512:1024
                    sc = SCP.tile([128, 1024], f32, tag="sc", name="sc")
                    nc.tensor.matmul(
                        sc[:, 0:512],
                        kts[h_lo][:, t * 128 : (t + 1) * 128],
                        qts[h_lo][:, j_lo * 512 : (j_lo + 1) * 512],
                        start=True,
                        stop=True,
                    )
                    nc.tensor.matmul(
                        sc[:, 512:1024],
                        kts[h_hi][:, t * 128 : (t + 1) * 128],
                        qts[h_hi][:, j_hi * 512 : (j_hi + 1) * 512],
                        start=True,
                        stop=True,
                    )
                    pt = WK.tile([128, 1024], bf16, tag="pt", name="pt")
                    nc.scalar.activation(pt[:], sc[:], EXP, scale=0.125)
                    pts.append(pt)
                    if t == 0 and pending_final is not None:
                        pending_final()
                        pending_final = None
                    emit_out_stage()
                    if t == 0:
                        emit_out_stage()  # free both cx banks right away
                    # interleave projections/V into the activation-bound
                    # steady state (after the exp emission so scores are
                    # never delayed behind projection work)
                    if pb_idx == 0:
                        if t == 0:
                            v_unit(0)
                        if t + 1 <= NT - 1:
                            v_unit(t + 1)
                        proj_step()
                        if t <= 21:
                            proj_step()
                    else:
                        proj_step()
                    # ctx runs one chunk behind exp so the PE overlaps the
                    # activation latency with the previous chunk's ctx
                    if t > 0:
                        emit_ctx(t - 1)
                # final chunk's ctx is deferred into the next block so the
                # transition never stalls on the last exp
                pending_final = (lambda f=emit_ctx: f(NT - 1))
                # mid-kernel transitions keep tp2 on the "pv" slot (free
                # outside pair-block 0) so proj "ps" chains are never blocked;
                # the final pair can use both slots and fully overlap
                tag2 = "ps" if pb_idx == len(PBS) - 1 else "pv"
                out_stage_q.append(
                    (j_lo, h_lo, cx_lo,
                     {"step": 0, "tag": "pv", "si": 2 * pb_idx})
                )
                out_stage_q.append(
                    (j_hi, h_hi, cx_hi,
                     {"step": 0, "tag": tag2, "si": 2 * pb_idx + 1})
                )
            if pending_final is not None:
                pending_final()
                pending_final = None
            flush_out_stages()

    nc.compile()
    return nc


def _get_nc():
    if "nc" not in _CACHE:
        _CACHE["nc"] = _build()
    return _CACHE["nc"]


def _in_maps(hs, mask, Wq, bq, Wk, bk, Wv, bv):
    ident = np.eye(128, dtype=np.float32)
    bones = np.zeros((128, 128), bf16np)
    bones[0, :] = 1.0

    def qk_chunks(W, hg):  # [768, :] f32 -> [128, 6*256] bf16: [h0|h1|h2|0]
        out = np.zeros((128, NHC * WCC), bf16np)
        for c in range(NHC):
            out[:, c * WCC : c * WCC + CC] = W[
                c * 128 : (c + 1) * 128, hg * CC : (hg + 1) * CC
            ].astype(bf16np)
        return out

    def v_chunks(W):  # augmented V weights -> [128, 6*195] bf16
        out = np.empty((128, NHC * VC), bf16np)
        for c in range(NHC):
            out[:, c * VC : (c + 1) * VC] = W[c * 128 : (c + 1) * 128, :].astype(
                bf16np
            )
        return out

    # per query-half: key order permuted so own queries are keys 0:2048.
    # hsT is block-interleaved: [p, b*3072 + c*512 + s] = hs.T[c*128+p, b*512+s]
    m32 = mask.reshape(NT, 128)
    hsT_sh = []
    maskt_sh = []
    for sh in range(QS):
        perm = np.roll(np.arange(S), -sh * SQ)
        a = hs[perm, :].astype(bf16np).T.reshape(NHC, 128, S // 512, 512)
        hsT_sh.append(
            np.ascontiguousarray(
                a.transpose(1, 2, 0, 3).reshape(128, NHC * S)
            )
        )
        maskt_sh.append(
            np.ascontiguousarray(np.roll(m32, -sh * (NT // QS), axis=0).T)
        )

    maps = []
    for core in range(N_CORES):
        hg, sh = core // QS, core % QS
        wv_aug = np.zeros((HID, VC), np.float32)
        bv_aug = np.zeros((128, VC), np.float32)
        for h in range(HPC):
            wv_aug[:, h * 65 : h * 65 + 64] = Wv[
                :, hg * CC + h * 64 : hg * CC + (h + 1) * 64
            ]
            bv_aug[0, h * 65 : h * 65 + 64] = bv[
                hg * CC + h * 64 : hg * CC + (h + 1) * 64
            ]
            bv_aug[0, h * 65 + 64] = 1.0
        # per-head bias columns: col 0 = h0 (lower half), col 1 = h1 (upper
        # half), col 2 = h2 (lower half; upper stays zero like its weights)
        bqt = np.zeros((128, HPC), np.float32)
        bkt = np.zeros((128, HPC), np.float32)
        for h, lo in ((0, 0), (1, 64), (2, 0)):
            bqt[lo : lo + 64, h] = bq[hg * CC + h * 64 : hg * CC + (h + 1) * 64]
            bkt[lo : lo + 64, h] = bk[hg * CC + h * 64 : hg * CC + (h + 1) * 64]
        maps.append(
            {
                "hsT": hsT_sh[sh],
                "wqb": qk_chunks(Wq, hg),
                "wkb": qk_chunks(Wk, hg),
                "wvb": v_chunks(wv_aug),
                "bqt": bqt,
                "bkt": bkt,
                "bvb": bv_aug.astype(bf16np),
                "bones": bones,
                "maskt": maskt_sh[sh],
                "ident": ident,
            }
        )
    return maps


def kernel(hidden_states, attention_mask, Wq, bq, Wk, bk, Wv, bv, **run_kwargs):
    hs = np.ascontiguousarray(np.asarray(hidden_states, np.float32).reshape(S, HID))
    mask = np.ascontiguousarray(np.asarray(attention_mask, np.float32).reshape(S))
    Wq = np.asarray(Wq, np.float32)
    Wk = np.asarray(Wk, np.float32)
    Wv = np.asarray(Wv, np.float32)
    bq = np.asarray(bq, np.float32)
    bk = np.asarray(bk, np.float32)
    bv = np.asarray(bv, np.float32)

    nc = _get_nc()
    maps = _in_maps(hs, mask, Wq, bq, Wk, bk, Wv, bv)
    res = bass_utils.run_bass_kernel_spmd(
        nc, maps, core_ids=list(range(N_CORES)), **run_kwargs
    )
    out = np.zeros((S, NH * HD), np.float32)
    for core in range(N_CORES):
        hg, sh = core // QS, core % QS
        raw = res.results[core]["out"].reshape(2 * len(PBS), 128, 4, 64)
        for pb_idx, (h_lo, h_hi, j_lo, j_hi) in enumerate(PBS):
            for k, (h, jq) in enumerate(((h_lo, j_lo), (h_hi, j_hi))):
                blk = raw[2 * pb_idx + k].transpose(1, 0, 2).reshape(512, 64)
                out[
                    sh * SQ + jq * 512 : sh * SQ + (jq + 1) * 512,
                    hg * CC + h * 64 : hg * CC + (h + 1) * 64,
                ] = blk
    if "trace" in run_kwargs:
        _CACHE["last_result"] = res
    return out.reshape(B, S, NH * HD)


# revision 59
# speedup vs baseline: 1.0017x; 1.0017x over previous
"""Trainium2 Bass kernel for BertSelfAttention (B=1, S=4096, HID=768, 12 heads).

Sharding: 8 cores = 4 head-groups x 2 query-halves. Each core computes 3 heads
for 2048 query rows against all 4096 keys, fused (scores never hit HBM).

v3 design (vs v1 baseline):
  - Two head-blocks are processed as a PAIR, key chunk by key chunk: one
    [128,1024] fp32 PSUM score tile holds BOTH blocks' scores for a chunk
    (lo block cols 0:512, hi block 512:1024) so ONE ScalarE exp per chunk
    covers both blocks. ScalarE (the bottleneck engine, ~1.15us per exp)
    runs back-to-back with zero idle in steady state.
  - Heads 0/1 pair per query block; head 2 pairs with itself across two
    query blocks. Score matmuls stay plain full-128 contractions with
    zero-padded halves: a row-tiled 64x128 variant was measured FASTER on
    the PE but pushed the chip into the P0 power state (all clocks x5/6),
    slowing the bottleneck ScalarE - zero halves toggle nothing and are
    power-cheap.
  - hsqT input dropped: per-core hsT is key-permuted so the core's own query
    rows are always columns 0:2048 (softmax is permutation-invariant over
    keys); q-projections just index hsT.
  - PSUM: sc 2x2 banks + cx_lo + cx_hi + 2 proj banks = 8 exactly.
  - kt/qt zero halves are written just-in-time by small DVE memsets per
    projection unit (no big ramp memsets).

Per-core dataflow otherwise follows v1: bf16 matmuls, fp32 PSUM, additive
mask handled by scaling V rows (and the appended ones-column) with exp(mask),
V augmented with a ones column per head so the context matmul accumulates the
softmax denominator for free, ctx^T tiles PE-transposed back to [q, d] and
divided by the denominator on VectorE.
"""

import sys

sys.path.insert(0, "/opt/trn_rl_repo")

import ml_dtypes
import numpy as np

import concourse.bacc as bacc
import concourse.mybir as mybir
import concourse.tile as tile
from concourse import bass_utils

B, S, HID = 1, 4096, 768
NH, HD = 12, 64
N_CORES = 8
HG = 4  # head-groups (tensor parallel)
QS = 2  # query splits (data parallel on sequence)
HPC = NH // HG  # 3 heads per core
SQ = S // QS  # 2048 query rows per core
CC = HPC * HD  # 192 projection columns per core
WCC = 256  # weight cols per chunk in wqb/wkb: [h0|h1|h2|h2]
VC = HPC * (HD + 1)  # 195 augmented V columns (ones col per head)
NHC = HID // 128  # 6 contraction chunks
NT = S // 128  # 32 key tiles
NJ = SQ // 512  # 4 query blocks

f32 = mybir.dt.float32
bf16 = mybir.dt.bfloat16
bf16np = ml_dtypes.bfloat16

# pair-blocks: (h_lo, h_hi, j_lo, j_hi) — heads 0/1 pair per query block,
# head 2 pairs with itself across two query blocks
PBS = [(0, 1, j, j) for j in range(NJ)] + [(2, 2, 0, 1), (2, 2, 2, 3)]

_CACHE = {}


def _build():
    EXP = mybir.ActivationFunctionType.Exp
    nc = bacc.Bacc("TRN2", target_bir_lowering=False)

    # hsT is block-interleaved host-side: [128 partitions, 8 query/key-column
    # blocks x (6 hid-chunks x 512 cols)] so every DMA slice is one fully
    # contiguous per-partition run (max packet size, ~10x queue throughput
    # vs the naive [HID, S] layout whose runs were 1KB strided)
    hsT_d = nc.dram_tensor("hsT", [128, NHC * S], bf16, kind="ExternalInput")
    wqb_d = nc.dram_tensor("wqb", [128, NHC * WCC], bf16, kind="ExternalInput")
    wkb_d = nc.dram_tensor("wkb", [128, NHC * WCC], bf16, kind="ExternalInput")
    wvb_d = nc.dram_tensor("wvb", [128, NHC * VC], bf16, kind="ExternalInput")
    bqt_d = nc.dram_tensor("bqt", [128, HPC], f32, kind="ExternalInput")
    bkt_d = nc.dram_tensor("bkt", [128, HPC], f32, kind="ExternalInput")
    bvb_d = nc.dram_tensor("bvb", [128, VC], bf16, kind="ExternalInput")
    bones_d = nc.dram_tensor("bones", [128, 128], bf16, kind="ExternalInput")
    maskt_d = nc.dram_tensor("maskt", [128, NT], f32, kind="ExternalInput")
    ident_d = nc.dram_tensor("ident", [128, 128], f32, kind="ExternalInput")
    # one contiguous [128, 256] slab per out-stage (2 per pair-block); the
    # host undoes the layout. Contiguous runs keep the out DMAs off the
    # slow small-packet path.
    out_d = nc.dram_tensor("out", [2 * len(PBS) * 128, 4 * 64], f32,
                           kind="ExternalOutput")

    with tile.TileContext(nc) as tc:
        with (
            tc.tile_pool(name="persist", bufs=1) as P,
            tc.tile_pool(name="work", bufs=4) as WK,
            tc.tile_pool(name="outp", bufs=2) as OP,
            tc.tile_pool(name="scp", bufs=2, space="PSUM") as SCP,
            tc.tile_pool(name="cxp", bufs=1, space="PSUM") as CP,
            tc.tile_pool(name="ppsum", bufs=1, space="PSUM") as PP,
        ):
            # ---- persistent SBUF tensors ----
            # chunk-major transposed activations: chunk c at cols [c*S, (c+1)*S)
            hsT = P.tile([128, NHC * S], bf16, tag="hsT")
            wqb = P.tile([128, NHC * WCC], bf16, tag="wqb")
            wkb = P.tile([128, NHC * WCC], bf16, tag="wkb")
            wvb = P.tile([128, NHC * VC], bf16, tag="wvb")
            bvb = P.tile([128, VC], bf16, tag="bvb")
            bones = P.tile([128, 128], bf16, tag="bones")
            bqt = P.tile([128, HPC], f32, tag="bqt")
            bkt = P.tile([128, HPC], f32, tag="bkt")
            maskt = P.tile([128, NT], f32, tag="maskt")
            wmask = P.tile([128, NT], f32, tag="wmask")
            identf = P.tile([128, 128], f32, tag="identf")
            # per-head K^T/Q^T: head h occupies partitions H_LO[h]:H_LO[h]+64,
            # the other half is zero (written by the zero-padded projection,
            # no memsets) so full-128 score contractions are exact and
            # power-cheap (the zero half toggles nothing in the PE array)
            kts = [
                P.tile([128, S], bf16, tag=f"kt{h}", name=f"kt{h}")
                for h in range(HPC)
            ]
            qts = [
                P.tile([128, SQ], bf16, tag=f"qt{h}", name=f"qt{h}")
                for h in range(HPC)
            ]
            vv = P.tile([128, NT * VC], bf16, tag="vv")

            # ---- DMA helpers ----
            HB = NHC * 512  # one 512-col block of all 6 chunks

            def load_hsT_block(b, queue="sync"):
                eng = nc.sync if queue == "sync" else nc.scalar
                eng.dma_start(
                    hsT[:, b * HB : (b + 1) * HB], hsT_d[:, b * HB : (b + 1) * HB]
                )

            # ---- q/k projection units ----
            # one paired matmul chain produces both partition halves:
            # pair 0 -> stationary cols 0:128 of each chunk ([h0|h1]),
            # pair 1 -> cols 128:256 ([h2|h2])
            def emit_qk_mm(kind, pi, j, c, ps):
                wsrc = wqb if kind == "qt" else wkb
                coff = 128 * pi
                nc.tensor.matmul(
                    ps[:],
                    wsrc[:, c * WCC + coff : c * WCC + coff + 128],
                    hsT[:, j * HB + c * 512 : j * HB + (c + 1) * 512],
                    start=(c == 0),
                    stop=(c == NHC - 1),
                )

            def emit_qk_finish(kind, pi, j, ps):
                dsts = qts if kind == "qt" else kts
                bias = bqt if kind == "qt" else bkt
                blk = slice(j * 512, (j + 1) * 512)
                if pi == 0:
                    nc.vector.tensor_scalar_add(
                        dsts[0][0:64, blk], ps[0:64, :], bias[0:64, 0:1]
                    )
                    nc.vector.tensor_scalar_add(
                        dsts[1][64:128, blk], ps[64:128, :], bias[64:128, 1:2]
                    )
                    # zero the complementary halves just-in-time (power-cheap
                    # full-128 contraction needs them zero)
                    nc.vector.memset(dsts[0][64:128, blk], 0.0)
                    nc.vector.memset(dsts[1][0:64, blk], 0.0)
                else:
                    # h2: upper weight cols are zero-padded, so the upper add
                    # writes zeros (+ zero bias) — both halves covered
                    nc.vector.tensor_scalar_add(
                        dsts[2][0:64, blk], ps[0:64, :], bias[0:64, 2:3]
                    )
                    nc.vector.tensor_scalar_add(
                        dsts[2][64:128, blk], ps[64:128, :], bias[64:128, 2:3]
                    )

            def qk_unit(kind, pi, j):
                ps = PP.tile([128, 512], f32, tag="ps", name="ps")
                for c in range(NHC):
                    emit_qk_mm(kind, pi, j, c, ps)
                emit_qk_finish(kind, pi, j, ps)

            # stepwise projection queue: one matmul per call so bursts never
            # overrun the per-tile PE slack
            proj_q = []

            def enqueue_proj(kind, pi, j):
                proj_q.append({"kind": kind, "pi": pi, "j": j, "step": 0})

            def proj_step():
                if not proj_q:
                    return
                st = proj_q[0]
                c = st["step"]
                if c == 0:
                    st["ps"] = PP.tile([128, 512], f32, tag="ps", name="ps")
                emit_qk_mm(st["kind"], st["pi"], st["j"], c, st["ps"])
                if c == NHC - 1:
                    emit_qk_finish(st["kind"], st["pi"], st["j"], st["ps"])
                    proj_q.pop(0)
                else:
                    st["step"] += 1

            def v_unit(t):
                pv = PP.tile([128, VC], f32, tag="pv", name="pv")
                base = (t // 4) * HB + (t % 4) * 128
                for c in range(NHC):
                    nc.tensor.matmul(
                        pv[:],
                        hsT[:, base + c * 512 : base + c * 512 + 128],
                        wvb[:, c * VC : (c + 1) * VC],
                        start=(c == 0),
                        stop=False,
                    )
                # bias add via row-0-selector stationary: full-128 operands so
                # the PE never leaves 128x128 tiling mode
                nc.tensor.matmul(pv[:], bones[:], bvb[:], start=False, stop=True)
                nc.vector.tensor_scalar_mul(
                    vv[:, t * VC : (t + 1) * VC], pv[:], wmask[:, t : t + 1]
                )

            # ---- deferred out-stage, pipelined into the next block ----
            out_stage_q = []

            def emit_out_stage():
                if not out_stage_q:
                    return
                # prioritize step-0 (the DVE copy that frees the cx PSUM
                # bank) of every queued entry, so the next block's ctx
                # accumulation never waits long on the bank
                entry = None
                for e in out_stage_q:
                    if e[3]["step"] == 0:
                        entry = e
                        break
                if entry is None:
                    entry = out_stage_q[0]
                _advance_out_stage(entry)

            def _advance_out_stage(entry):
                jq, h, cx, st = entry
                if st["step"] == 0:
                    cs = OP.tile([65, 512], f32, tag="cs", name="cs")
                    nc.vector.tensor_copy(cs[:], cx[:])
                    st["cs"] = cs
                    st["ot"] = OP.tile([128, 4 * 64], f32, tag="ot", name="ot")
                elif st["step"] == 1:
                    cs = st["cs"]
                    tp2 = PP.tile(
                        [128, 4 * 65], f32, tag=st.get("tag", "pv"), name="tp2"
                    )
                    st["tp2"] = tp2
                    for t4 in range(4):
                        nc.tensor.transpose(
                            tp2[:, t4 * 65 : (t4 + 1) * 65],
                            cs[:, t4 * 128 : (t4 + 1) * 128],
                            identf[0:65, 0:65],
                        )
                elif st["step"] == 2:
                    tp2 = st["tp2"]
                    # one strided reciprocal covers all four denominator cols
                    rc = OP.tile([128, 4], f32, tag="rc", name="rc")
                    den = tp2.rearrange("p (t c) -> p t c", c=65)[:, :, 64:65]
                    nc.vector.reciprocal(rc.rearrange("p (t c) -> p t c", c=1), den)
                    st["rc"] = rc
                    nc.vector.tensor_scalar_mul(
                        st["ot"][:, 0:64], tp2[:, 0:64], rc[:, 0:1]
                    )
                    nc.vector.tensor_scalar_mul(
                        st["ot"][:, 64:128], tp2[:, 65:129], rc[:, 1:2]
                    )
                elif st["step"] == 3:
                    tp2, ot, rc = st["tp2"], st["ot"], st["rc"]
                    nc.vector.tensor_scalar_mul(
                        ot[:, 128:192], tp2[:, 130:194], rc[:, 2:3]
                    )
                    nc.vector.tensor_scalar_mul(
                        ot[:, 192:256], tp2[:, 195:259], rc[:, 3:4]
                    )
                    si = st["si"]
                    # the final pair's outputs go out on the fast scalar
                    # queue (ScalarE is idle by then); mid-kernel stages use
                    # sync so DMA pushes never occupy the bottleneck engine
                    eng = nc.scalar if st.get("tag") == "ps" else nc.sync
                    eng.dma_start(out_d[si * 128 : (si + 1) * 128, :], ot[:])
                    for idx, e in enumerate(out_stage_q):
                        if e[3] is st:
                            del out_stage_q[idx]
                            break
                    return
                st["step"] += 1

            def flush_out_stages():
                # round-robin so the two final out-stages (on separate PSUM
                # slots) overlap across engines
                while out_stage_q:
                    for e in list(out_stage_q):
                        _advance_out_stage(e)

            # ---- ramp: pipelined input loads + first-needed projections ----
            # mask load + exp first: ScalarE is in-order, so this tiny
            # ACTIVATE must clear the queue before the first score exp
            # minimal ramp: only what gates the first score exp. Everything
            # else is JIT inside pair-block 0, where the activation stream
            # covers ~1.1us of PE work per tile; ramp work has zero overlap.
            # the first hsT block rides the scalar HWDGE queue ahead of the
            # mask activation so it lands as early as possible
            # wqb first on the fast scalar queue: it gates all PE work
            nc.scalar.dma_start(wqb[:], wqb_d[:])
            load_hsT_block(0, queue="scalar")
            load_hsT_block(1, queue="scalar")
            nc.sync.dma_start(maskt[:], maskt_d[:])
            nc.sync.dma_start(bqt[:], bqt_d[:])
            nc.sync.dma_start(wkb[:], wkb_d[:])
            nc.sync.dma_start(bkt[:], bkt_d[:])
            nc.scalar.activation(wmask[:], maskt[:], EXP)
            nc.sync.dma_start(wvb[:], wvb_d[:])
            nc.sync.dma_start(bvb[:], bvb_d[:])
            nc.sync.dma_start(bones[:], bones_d[:])
            nc.sync.dma_start(identf[:], ident_d[:])
            load_hsT_block(2)
            load_hsT_block(3)
            # short PE warm-up (gated only on the wqb DMA) to open the HAM
            # clock gate before the first projections
            warm = PP.tile([128, 512], f32, tag="ps", name="warm")
            for i in range(5):
                nc.tensor.matmul(
                    warm[:],
                    wqb[:, 0:128],
                    wqb[:, 512:1024],
                    start=True,
                    stop=True,
                )
            qk_unit("qt", 0, 0)
            # kt block 0 in two pieces: keys 0:128 first, so the first score
            # matmul (which only needs key chunk 0) fires ~2us earlier
            ps0 = PP.tile([128, 512], f32, tag="ps", name="ps0")
            for piece in ((0, 128), (128, 512)):
                p0, p1 = piece
                for c in range(NHC):
                    nc.tensor.matmul(
                        ps0[:, p0:p1],
                        wkb[:, c * WCC : c * WCC + 128],
                        hsT[:, c * 512 + p0 : c * 512 + p1],
                        start=(c == 0),
                        stop=(c == NHC - 1),
                    )
                nc.vector.tensor_scalar_add(
                    kts[0][0:64, p0:p1], ps0[0:64, p0:p1], bkt[0:64, 0:1]
                )
                nc.vector.tensor_scalar_add(
                    kts[1][64:128, p0:p1], ps0[64:128, p0:p1], bkt[64:128, 1:2]
                )
                nc.vector.memset(kts[0][64:128, p0:p1], 0.0)
                nc.vector.memset(kts[1][0:64, p0:p1], 0.0)
            load_hsT_block(4, queue="scalar")
            load_hsT_block(5, queue="scalar")
            load_hsT_block(6)
            load_hsT_block(7)

            # per-pair-block projection enqueue schedule (ready just in time)
            pb_enqueue = {
                0: [("kt", 0, j) for j in range(1, 8)] + [("qt", 0, 1)],
                1: [("qt", 0, 2)] + [("kt", 1, j) for j in range(4)],
                2: [("qt", 0, 3)] + [("kt", 1, j) for j in range(4, 8)],
                3: [("qt", 1, 0), ("qt", 1, 1)],
                4: [("qt", 1, 2), ("qt", 1, 3)],
            }

            pending_final = None

            for pb_idx, (h_lo, h_hi, j_lo, j_hi) in enumerate(PBS):
                for item in pb_enqueue.get(pb_idx, []):
                    enqueue_proj(*item)
                cx_lo = CP.tile([65, 512], f32, tag="cxlo", name="cxlo")
                cx_hi = CP.tile([65, 512], f32, tag="cxhi", name="cxhi")
                pts = []

                def emit_ctx(g, pts=pts, cx_lo=cx_lo, cx_hi=cx_hi,
                             h_lo=h_lo, h_hi=h_hi):
                    pt = pts[g]
                    nc.tensor.matmul(
                        cx_lo[:],
                        vv[:, g * VC + h_lo * 65 : g * VC + h_lo * 65 + 65],
                        pt[:, 0:512],
                        start=(g == 0),
                        stop=(g == NT - 1),
                    )
                    nc.tensor.matmul(
                        cx_hi[:],
                        vv[:, g * VC + h_hi * 65 : g * VC + h_hi * 65 + 65],
                        pt[:, 512:1024],
                        start=(g == 0),
                        stop=(g == NT - 1),
                    )

                for t in range(NT):
                    # scores for key chunk t, both paired head-blocks, as
                    # plain full-128 contractions (zero halves contribute
                    # nothing): lo block -> cols 0:512, hi block -> 512:1024
                    sc = SCP.tile([128, 1024], f32, tag="sc", name="sc")
                    if pb_idx == 0:
                        # pair-block 0 is PE-overloaded (all the JIT
                        # projection work); run its score pair row-tiled
                        # 64x128 — h0 lives on partitions 0:64, h1 on
                        # 64:128, so the two matmuls execute concurrently
                        # on the two PE row-halves. Short burst only: a
                        # whole kernel of this trips the P0 power downclock.
                        nc.tensor.matmul(
                            sc[:, 0:512],
                            kts[h_lo][0:64, t * 128 : (t + 1) * 128],
                            qts[h_lo][0:64, j_lo * 512 : (j_lo + 1) * 512],
                            start=True,
                            stop=True,
                        )
                        nc.tensor.matmul(
                            sc[:, 512:1024],
                            kts[h_hi][64:128, t * 128 : (t + 1) * 128],
                            qts[h_hi][64:128, j_hi * 512 : (j_hi + 1) * 512],
                            start=True,
                            stop=True,
                        )
                    else:
                        nc.tensor.matmul(
                            sc[:, 0:512],
                            kts[h_lo][:, t * 128 : (t + 1) * 128],
                            qts[h_lo][:, j_lo * 512 : (j_lo + 1) * 512],
                            start=True,
                            stop=True,
                        )
                        nc.tensor.matmul(
                            sc[:, 512:1024],
                            kts[h_hi][:, t * 128 : (t + 1) * 128],
                            qts[h_hi][:, j_hi * 512 : (j_hi + 1) * 512],
                            start=True,
                            stop=True,
                        )
                    pt = WK.tile([128, 1024], bf16, tag="pt", name="pt")
                    nc.scalar.activation(pt[:], sc[:], EXP, scale=0.125)
                    pts.append(pt)
                    if t == 0 and pending_final is not None:
                        pending_final()
                        pending_final = None
                    emit_out_stage()
                    if t == 0:
                        emit_out_stage()  # free both cx banks right away
                    # interleave projections/V into the activation-bound
                    # steady state (after the exp emission so scores are
                    # never delayed behind projection work)
                    if pb_idx == 0:
                        if t == 0:
                            v_unit(0)
                        if t + 1 <= NT - 1:
                            v_unit(t + 1)
                        proj_step()
                        if t <= 21:
                            proj_step()
                    else:
                        proj_step()
                    # ctx runs one chunk behind exp so the PE overlaps the
                    # activation latency with the previous chunk's ctx
                    if t > 0:
                        emit_ctx(t - 1)
                # final chunk's ctx is deferred into the next block so the
                # transition never stalls on the last exp
                pending_final = (lambda f=emit_ctx: f(NT - 1))
                # mid-kernel transitions keep tp2 on the "pv" slot (free
                # outside pair-block 0) so proj "ps" chains are never blocked;
                # the final pair can use both slots and fully overlap
                tag2 = "ps" if pb_idx == len(PBS) - 1 else "pv"
                out_stage_q.append(
                    (j_lo, h_lo, cx_lo,
                     {"step": 0, "tag": "pv", "si": 2 * pb_idx})
                )
                out_stage_q.append(
                    (j_hi, h_hi, cx_hi,
                     {"step": 0, "tag": tag2, "si": 2 * pb_idx + 1})
                )
            if pending_final is not None:
                pending_final()
                pending_final = None
            flush_out_stages()

    nc.compile()
    return nc


def _get_nc():
    if "nc" not in _CACHE:
        _CACHE["nc"] = _build()
    return _CACHE["nc"]


def _in_maps(hs, mask, Wq, bq, Wk, bk, Wv, bv):
    ident = np.eye(128, dtype=np.float32)
    bones = np.zeros((128, 128), bf16np)
    bones[0, :] = 1.0

    def qk_chunks(W, hg):  # [768, :] f32 -> [128, 6*256] bf16: [h0|h1|h2|0]
        out = np.zeros((128, NHC * WCC), bf16np)
        for c in range(NHC):
            out[:, c * WCC : c * WCC + CC] = W[
                c * 128 : (c + 1) * 128, hg * CC : (hg + 1) * CC
            ].astype(bf16np)
        return out

    def v_chunks(W):  # augmented V weights -> [128, 6*195] bf16
        out = np.empty((128, NHC * VC), bf16np)
        for c in range(NHC):
            out[:, c * VC : (c + 1) * VC] = W[c * 128 : (c + 1) * 128, :].astype(
                bf16np
            )
        return out

    # per query-half: key order permuted so own queries are keys 0:2048.
    # hsT is block-interleaved: [p, b*3072 + c*512 + s] = hs.T[c*128+p, b*512+s]
    m32 = mask.reshape(NT, 128)
    hsT_sh = []
    maskt_sh = []
    for sh in range(QS):
        perm = np.roll(np.arange(S), -sh * SQ)
        a = hs[perm, :].astype(bf16np).T.reshape(NHC, 128, S // 512, 512)
        hsT_sh.append(
            np.ascontiguousarray(
                a.transpose(1, 2, 0, 3).reshape(128, NHC * S)
            )
        )
        maskt_sh.append(
            np.ascontiguousarray(np.roll(m32, -sh * (NT // QS), axis=0).T)
        )

    maps = []
    for core in range(N_CORES):
        hg, sh = core // QS, core % QS
        wv_aug = np.zeros((HID, VC), np.float32)
        bv_aug = np.zeros((128, VC), np.float32)
        for h in range(HPC):
            wv_aug[:, h * 65 : h * 65 + 64] = Wv[
                :, hg * CC + h * 64 : hg * CC + (h + 1) * 64
            ]
            bv_aug[0, h * 65 : h * 65 + 64] = bv[
                hg * CC + h * 64 : hg * CC + (h + 1) * 64
            ]
            bv_aug[0, h * 65 + 64] = 1.0
        # per-head bias columns: col 0 = h0 (lower half), col 1 = h1 (upper
        # half), col 2 = h2 (lower half; upper stays zero like its weights)
        bqt = np.zeros((128, HPC), np.float32)
        bkt = np.zeros((128, HPC), np.float32)
        for h, lo in ((0, 0), (1, 64), (2, 0)):
            bqt[lo : lo + 64, h] = bq[hg * CC + h * 64 : hg * CC + (h + 1) * 64]
            bkt[lo : lo + 64, h] = bk[hg * CC + h * 64 : hg * CC + (h + 1) * 64]
        maps.append(
            {
                "hsT": hsT_sh[sh],
                "wqb": qk_chunks(Wq, hg),
                "wkb": qk_chunks(Wk, hg),
                "wvb": v_chunks(wv_aug),
                "bqt": bqt,
                "bkt": bkt,
                "bvb": bv_aug.astype(bf16np),
                "bones": bones,
                "maskt": maskt_sh[sh],
                "ident": ident,
            }
        )
    return maps


def kernel(hidden_states, attention_mask, Wq, bq, Wk, bk, Wv, bv, **run_kwargs):
    hs = np.ascontiguousarray(np.asarray(hidden_states, np.float32).reshape(S, HID))
    mask = np.ascontiguousarray(np.asarray(attention_mask, np.float32).reshape(S))
    Wq = np.asarray(Wq, np.float32)
    Wk = np.asarray(Wk, np.float32)
    Wv = np.asarray(Wv, np.float32)
    bq = np.asarray(bq, np.float32)
    bk = np.asarray(bk, np.float32)
    bv = np.asarray(bv, np.float32)

    nc = _get_nc()
    maps = _in_maps(hs, mask, Wq, bq, Wk, bk, Wv, bv)
    res = bass_utils.run_bass_kernel_spmd(
        nc, maps, core_ids=list(range(N_CORES)), **run_kwargs
    )
    out = np.zeros((S, NH * HD), np.float32)
    for core in range(N_CORES):
        hg, sh = core // QS, core % QS
        raw = res.results[core]["out"].reshape(2 * len(PBS), 128, 4, 64)
        for pb_idx, (h_lo, h_hi, j_lo, j_hi) in enumerate(PBS):
            for k, (h, jq) in enumerate(((h_lo, j_lo), (h_hi, j_hi))):
                blk = raw[2 * pb_idx + k].transpose(1, 0, 2).reshape(512, 64)
                out[
                    sh * SQ + jq * 512 : sh * SQ + (jq + 1) * 512,
                    hg * CC + h * 64 : hg * CC + (h + 1) * 64,
                ] = blk
    if "trace" in run_kwargs:
        _CACHE["last_result"] = res
    return out.reshape(B, S, NH * HD)
